# revision 20
# baseline (speedup 1.0000x reference)
"""Mamba-1 selective scan on 8 Trainium2 NeuronCores — n-in-free-dim design.

Sharding: core c -> (batch b = c//2, D-half h = c%2): each core owns 512
channels of one batch for the recurrence; projections need the full D=1024.

Math (exact ZOH, rescaled state):
  G = A + 1e-8,  shat := G * s
  a_t = exp(dt_t * A)                           (per d,n,t)
  shat_t = a_t shat_{t-1} + (a_t - 1) ghat_t,   ghat = x * B
  w := shat + ghat  ->  w_t = (delta_t + w_{t-1}) * a_t,
       delta_t = ghat_t - ghat_{t-1}            (hw tensor_tensor_scan)
  y_t[d] = sum_n (1/G)[d,n] (w - ghat) C[n,t] + Dskip[d] x[d,t]
         = [sum_n diag(1/G_n) @ (w_n * crep_n)]  - x*q + Dskip*x
    q[d,t] = sum_n (1/G)[d,n] B[n,t] C[n,t]     (PE matmul of bc = B*C)

On-chip layout: partitions = 128 channels (4 d-tiles per core); n and time
in the free dim: per (n, dtile, chunk) tiles [128, TC].  All elementwise in
fp16 for the DVE 2x mode; scans are 1x; a is fp16 (exp computed f32 on ACT).
xt/pall columns are time-shifted by +1 (col 0 = time -1 = zeros) so the
delta at chunk boundaries needs no carry.
"""

import sys

import numpy as np

sys.path.insert(0, "/opt/trn_rl_repo")

import concourse.bacc as bacc
import concourse.mybir as mybir
import concourse.tile as tile
from concourse.bass_utils import run_bass_kernel_spmd

B, T, D, N, R = 4, 4096, 1024, 16, 64
NCORES = 8
DH = D // 2            # channels per core
NDT = DH // 128        # d-tiles per core (4)
KD = D // 128          # k-tiles over full D for projections (8)
TC = 1024              # time chunk
NCH = T // TC
PH = 512               # psum piece (one bank of f32)
NPC = TC // PH         # psum pieces per chunk (2)
F32 = mybir.dt.float32
FP16 = mybir.dt.float16
AL = mybir.AluOpType
AF = mybir.ActivationFunctionType

# engine assignment for the sct multiply: give Pool every POOL_SCT-th tile
POOL_SCT = 2

_CACHE = {}


def _patch_act_tables():
    """Route Exp+Ln to natural_log_exp_and_others so the softplus (Exp,Ln)
    and the main-loop Exp never force activation-table reloads."""
    import concourse.bacc as _bacc
    from concourse.hw_specs import get_activation_tables as _orig

    def patched(arch):
        t = _orig(arch)
        exp = mybir.ActivationFunctionType.Exp
        ln = mybir.ActivationFunctionType.Ln
        for name, fns in t.items():
            if name != "natural_log_exp_and_others":
                fns.discard(exp)
                fns.discard(ln)
        return t

    _bacc.get_activation_tables = patched


def _build_program():
    _patch_act_tables()
    nc = bacc.Bacc(
        "TRN2",
        target_bir_lowering=False,
        debug=False,
        num_devices=NCORES,
    )

    x_d = nc.dram_tensor("x16", [T, D], FP16, kind="ExternalInput")
    wall_d = nc.dram_tensor("wall", [128, KD * 112], FP16, kind="ExternalInput")
    w2_d = nc.dram_tensor("w2r", [64, NDT * 128], FP16, kind="ExternalInput")
    bd_d = nc.dram_tensor("bdt2", [128, NDT], F32, kind="ExternalInput")
    ac_d = nc.dram_tensor("acols", [128, NDT * N], F32, kind="ExternalInput")
    dgw_d = nc.dram_tensor("dgw", [128, NDT * N * 128], FP16,
                           kind="ExternalInput")
    dsk_d = nc.dram_tensor("dskw", [128, NDT * 128], FP16,
                           kind="ExternalInput")
    qw_d = nc.dram_tensor("qw", [16, NDT * 128], FP16, kind="ExternalInput")
    sel_d = nc.dram_tensor("selbc", [128, 2 * N * 128], FP16, kind="ExternalInput")
    nid_d = nc.dram_tensor("nident", [128, 128], FP16, kind="ExternalInput")
    id16_d = nc.dram_tensor("ident16", [128, 128], FP16, kind="ExternalInput")
    w0_d = nc.dram_tensor("w0init", [128, NDT * N], F32, kind="ExternalInput")
    y_d = nc.dram_tensor("yT", [DH, T], FP16, kind="ExternalOutput")

    with tile.TileContext(nc) as tc:
        _body(tc, x_d, wall_d, w2_d, bd_d, ac_d, dgw_d, dsk_d, qw_d, sel_d,
              nid_d, id16_d, w0_d, y_d)

    nc.compile()
    return nc


def _body(tc, x_d, wall_d, w2_d, bd_d, ac_d, dgw_d, dsk_d, qw_d, sel_d,
          nid_d, id16_d, w0_d, y_d):
    nc = tc.nc

    with (
        tc.tile_pool(name="const", bufs=1) as const,
        tc.tile_pool(name="xload", bufs=2) as xload,
        tc.tile_pool(name="xtmp", bufs=1) as xtmpp,
        tc.tile_pool(name="xtc", bufs=2) as xtcp,
        tc.tile_pool(name="pallc", bufs=2) as pallcp,
        tc.tile_pool(name="bcache", bufs=1) as bcache,
        tc.tile_pool(name="dtp", bufs=2) as dtp,
        tc.tile_pool(name="work", bufs=1) as workp,
        tc.tile_pool(name="atp", bufs=2) as atp,
        tc.tile_pool(name="gwork", bufs=3) as gworkp,
        tc.tile_pool(name="scan", bufs=3) as scanp,
        tc.tile_pool(name="sctp", bufs=3) as sctp,
        tc.tile_pool(name="yout", bufs=1) as youtp,
        tc.tile_pool(name="psA", bufs=1, space="PSUM") as psA,
        tc.tile_pool(name="psB", bufs=2, space="PSUM") as psB,
        tc.tile_pool(name="psY", bufs=1, space="PSUM") as psY,
        tc.tile_pool(name="psT", bufs=2, space="PSUM") as psT,
    ):
        # ---- constants ----
        wall = const.tile([128, KD, 112], FP16)
        nc.sync.dma_start(wall, wall_d.ap().rearrange("p (k m) -> p k m",
                                                        k=KD))
        w2r = const.tile([64, NDT, 128], FP16)
        nc.scalar.dma_start(w2r, w2_d.ap().rearrange("p (d m) -> p d m",
                                                     d=NDT))
        bdt2 = const.tile([128, NDT], F32)
        nc.scalar.dma_start(bdt2, bd_d[:, :])
        acols = const.tile([128, NDT * N], F32)
        nc.scalar.dma_start(acols, ac_d[:, :])
        dgw = const.tile([128, NDT * N, 128], FP16)
        nc.gpsimd.dma_start(dgw, dgw_d.ap().rearrange("p (g m) -> p g m",
                                                      g=NDT * N))
        dskw = const.tile([128, NDT, 128], FP16)
        nc.sync.dma_start(dskw, dsk_d.ap().rearrange("p (d m) -> p d m",
                                                       d=NDT))
        qw = const.tile([16, NDT, 128], FP16)
        nc.sync.dma_start(qw, qw_d.ap().rearrange("p (d m) -> p d m",
                                                    d=NDT))
        selbc = const.tile([128, 2 * N, 128], FP16)
        nc.scalar.dma_start(selbc, sel_d.ap().rearrange("p (n m) -> p n m",
                                                        n=2 * N))
        nident = const.tile([128, 128], FP16)
        nc.scalar.dma_start(nident, nid_d[:, :])
        ident16 = const.tile([128, 128], FP16)
        nc.sync.dma_start(ident16, id16_d[:, :])
        wc = const.tile([128, NDT * N], F32)
        nc.gpsimd.dma_start(wc, w0_d[:, :])

        stage_prev = {}

        def stage_alloc(ch):
            t0 = ch * TC
            # xtc col j <-> time t0-1+j; pallc col j <-> time t0-1+j
            xtc = xtcp.tile([128, NDT, TC + 1], FP16, tag="xtc", name="xtc")
            pallc = pallcp.tile([112, TC + 1], FP16, tag="pallc",
                                name="pallc")
            if ch == 0:
                nc.vector.memset(xtc[:, :, 0:1], 0.0)
                nc.vector.memset(pallc[:, 0:1], 0.0)
            else:
                xp, pp0 = stage_prev[ch - 1]
                nc.scalar.copy(xtc[:, :, 0:1], xp[:, :, TC: TC + 1])
                nc.scalar.copy(pallc[:, 0:1], pp0[:, TC: TC + 1])
            stage_prev[ch] = (xtc, pallc)

        def stage_piece(ch, tp):
            t0 = ch * TC
            xtc, pallc = stage_prev[ch]
            xls = []
            for j in range(4):
                xld = xload.tile([128, D], FP16, tag=f"xld{j}",
                                 name=f"xld{j}")
                nc.sync.dma_start(
                    xld, x_d[t0 + tp * PH + j * 128:
                             t0 + tp * PH + (j + 1) * 128, :])
                xls.append(xld)
            ktiles = []
            for k in range(KD):
                ptr = psT.tile([128, PH], FP16, tag="psT")
                for j in range(4):
                    nc.tensor.transpose(
                        ptr[:, j * 128:(j + 1) * 128],
                        xls[j][:, k * 128:(k + 1) * 128], ident16)
                if k < NDT:
                    dst = xtc[:, k, 1 + tp * PH: 1 + (tp + 1) * PH]
                    nc.scalar.copy(dst, ptr)
                    ktiles.append(
                        xtc[:, k, 1 + tp * PH: 1 + (tp + 1) * PH])
                else:
                    xtm = xtmpp.tile([128, PH], FP16, tag=f"xtm{k}",
                                     name=f"xtm{k}")
                    nc.scalar.copy(xtm, ptr)
                    ktiles.append(xtm)
            pp = psT.tile([112, PH], F32, tag="psT")
            for k in range(KD):
                nc.tensor.matmul(pp, wall[:, k, :], ktiles[k],
                                 start=(k == 0), stop=(k == KD - 1))
            nc.scalar.copy(pallc[:, 1 + tp * PH: 1 + (tp + 1) * PH], pp)

        itercnt = 0
        stage_alloc(0)
        for _tp in range(NPC):
            stage_piece(0, _tp)
        for ch in range(NCH):
            t0 = ch * TC
            xtc, pallc = stage_prev[ch]

            # ---- bc = B*C for the q correction (copies realign base) ----
            btc = workp.tile([16, TC], FP16, tag="btc", name="btc")
            nc.scalar.copy(btc, pallc[64:80, 1: 1 + TC])
            ctc = workp.tile([16, TC], FP16, tag="ctc", name="ctc")
            nc.scalar.copy(ctc, pallc[96:112, 1: 1 + TC])
            bc = workp.tile([16, TC], FP16, tag="bc", name="bc")
            nc.vector.tensor_tensor(bc, btc, ctc, AL.mult)

            dts = dtp.tile([128, NDT, TC], FP16, tag="dts", name="dts")
            breps, creps = [None] * N, [None] * N

            # ---- recurrence per (dtile, n) ----
            for dtl in range(NDT):
                # dt for this dtile: softplus(w2 @ xr + b)
                for hf in range(NPC):
                    sl = slice(1 + hf * PH, 1 + (hf + 1) * PH)
                    pdt = psB.tile([128, PH], F32, tag="psB")
                    nc.tensor.matmul(pdt, w2r[:, dtl, :], pallc[0:64, sl],
                                     start=True, stop=True)
                    dsl = dts[:, dtl, hf * PH:(hf + 1) * PH]
                    nc.scalar.activation(dsl, pdt, AF.Exp,
                                         bias=bdt2[:, dtl:dtl + 1], scale=1.0)
                nc.scalar.activation(dts[:, dtl, :], dts[:, dtl, :],
                                     AF.Ln, bias=1.0, scale=1.0)

                py = psY.tile([128, TC], F32, tag="psY", name="py")
                pys = [py[:, hf * PH:(hf + 1) * PH] for hf in range(NPC)]
                # q correction (early; independent of the n loop):
                # ycr = x * (qw @ bc), subtracted from y via -I weights
                ycr = workp.tile([128, TC], FP16, tag="ycr", name="ycr")
                for hf in range(NPC):
                    pq = psB.tile([128, PH], F32, tag="psB")
                    nc.tensor.matmul(pq, qw[:, dtl, :],
                                     bc[:, hf * PH:(hf + 1) * PH],
                                     start=True, stop=True)
                    nc.vector.tensor_tensor(
                        ycr[:, hf * PH:(hf + 1) * PH],
                        xtc[:, dtl, 1 + hf * PH: 1 + (hf + 1) * PH],
                        pq, AL.mult)
                for n in range(N):
                    g = dtl * N + n
                    if dtl == 0:
                        # lazy broadcast of B and C rows to 128 partitions:
                        # brep1[n] covers times t0-1..t0+TC-1, crep[n]
                        # covers t0..t0+TC-1
                        br = bcache.tile([128, TC + 1], FP16, tag=f"br{n}",
                                         name=f"br{n}")
                        pb = psA.tile([128, TC], F32, tag="psA")
                        for hf in range(NPC):
                            nc.tensor.matmul(
                                pb[:, hf * PH:(hf + 1) * PH],
                                selbc[64:112, n, :],
                                pallc[64:112, hf * PH: (hf + 1) * PH],
                                start=True, stop=True, tile_position=(64, 0))
                        nc.scalar.copy(br[:, 0:TC], pb)
                        pbl = psB.tile([128, PH], F32, tag="psB")
                        nc.tensor.matmul(pbl[:, 0:1], selbc[64:112, n, :],
                                         pallc[64:112, TC: TC + 1],
                                         start=True, stop=True,
                                         tile_position=(64, 0))
                        nc.scalar.copy(br[:, TC: TC + 1], pbl[:, 0:1])
                        breps[n] = br

                        cr = bcache.tile([128, TC], FP16, tag=f"cr{n}",
                                         name=f"cr{n}")
                        pc = psA.tile([128, TC], F32, tag="psA")
                        for hf in range(NPC):
                            nc.tensor.matmul(
                                pc[:, hf * PH:(hf + 1) * PH],
                                selbc[64:112, N + n, :],
                                pallc[64:112, 1 + hf * PH: 1 + (hf + 1) * PH],
                                start=True, stop=True, tile_position=(64, 0))
                        nc.scalar.copy(cr, pc)
                        creps[n] = cr

                    at = atp.tile([128, TC], F32, tag="at", name="at")
                    nc.scalar.activation(at, dts[:, dtl, :], AF.Exp,
                                         scale=acols[:, g:g + 1])
                    gt = gworkp.tile([128, TC + 1], FP16, tag="gt", name="gt")
                    nc.vector.tensor_tensor(
                        gt, xtc[:, dtl, 0: TC + 1], breps[n], AL.mult)
                    dl = gworkp.tile([128, TC], FP16, tag="dl", name="dl")
                    nc.vector.tensor_tensor(
                        dl, gt[:, 1: TC + 1], gt[:, 0: TC], AL.subtract)
                    wt = scanp.tile([128, TC], FP16, tag="wt", name="wt")
                    nc.vector.tensor_tensor_scan(
                        wt, dl, at, wc[:, g:g + 1], op0=AL.add, op1=AL.mult)
                    nc.scalar.copy(wc[:, g:g + 1], wt[:, TC - 1: TC])
                    sct = sctp.tile([128, TC], FP16, tag="sct", name="sct")
                    nc.gpsimd.tensor_tensor(sct, wt, creps[n], AL.mult)
                    itercnt += 1
                    for hf in range(NPC):
                        nc.tensor.matmul(
                            pys[hf], dgw[:, g, :],
                            sct[:, hf * PH:(hf + 1) * PH],
                            start=(n == 0), stop=False)

                # D_skip * x
                for hf in range(NPC):
                    nc.tensor.matmul(
                        pys[hf], dskw[:, dtl, :],
                        xtc[:, dtl, 1 + hf * PH: 1 + (hf + 1) * PH],
                        start=False, stop=False)
                for hf in range(NPC):
                    nc.tensor.matmul(pys[hf], nident,
                                     ycr[:, hf * PH:(hf + 1) * PH],
                                     start=False, stop=(True))

                yo = youtp.tile([128, TC], FP16, tag="yo", name="yo")
                nc.scalar.copy(yo, py)
                nc.sync.dma_start(
                    y_d[dtl * 128:(dtl + 1) * 128, t0: t0 + TC], yo)

                # interleave next chunk's transposes+projections
                if ch + 1 < NCH:
                    if dtl == 0:
                        stage_alloc(ch + 1)
                        stage_piece(ch + 1, 0)
                    elif dtl == 1:
                        stage_piece(ch + 1, 1)


def kernel(x, state, log_A, W_B, W_C, W_dt1, W_dt2, b_dt2, D_skip):
    if "nc" not in _CACHE:
        _CACHE["nc"] = _build_program()
    nc = _CACHE["nc"]

    x = np.asarray(x, np.float32)
    state = np.asarray(state, np.float32)
    A = (-np.exp(np.asarray(log_A, np.float32))).astype(np.float32)
    G = (A + np.float32(1e-8)).astype(np.float32)
    invG = (np.float32(1.0) / G).astype(np.float32)
    W_B = np.asarray(W_B, np.float32)
    W_C = np.asarray(W_C, np.float32)
    W_dt1 = np.asarray(W_dt1, np.float32)
    W_dt2 = np.asarray(W_dt2, np.float32)
    b_dt2 = np.asarray(b_dt2, np.float32)
    D_skip = np.asarray(D_skip, np.float32)

    nident = (-np.eye(128)).astype(np.float16)
    ident16 = np.eye(128).astype(np.float16)
    selbc = np.zeros((128, 2 * N * 128), np.float16)
    for n in range(N):
        selbc[64 + n, n * 128:(n + 1) * 128] = 1.0        # B row n (part 64+n)
        selbc[96 + n, (N + n) * 128:(N + n + 1) * 128] = 1.0  # C row (96+n)

    in_maps = []
    for c in range(NCORES):
        b, h = c // 2, c % 2
        loc = slice(h * DH, (h + 1) * DH)
        perm = np.r_[np.arange(h * DH, (h + 1) * DH),
                     np.arange((1 - h) * DH, (2 - h) * DH)]
        Al = A[loc]                      # [DH, N]
        Gl = G[loc]
        invGl = invG[loc]

        # wall: [W_B.T | W_C.T | W_dt1.T] with permuted rows -> [128, KD*96]
        wallf = np.concatenate(
            [W_dt1.T[perm], W_B.T[perm],
             np.zeros((D, 16), np.float32), W_C.T[perm]], axis=1)  # [D, 112]
        wall = np.ascontiguousarray(
            wallf.reshape(KD, 128, 112).transpose(1, 0, 2).reshape(
                128, KD * 112)).astype(np.float16)

        # w2r: [64, NDT*128]
        w2r = np.ascontiguousarray(
            W_dt2[loc].T.reshape(64, NDT, 128).reshape(64, NDT * 128)
        ).astype(np.float16)

        bd = np.ascontiguousarray(b_dt2[loc].reshape(NDT, 128).T)

        # acols: [128, NDT*N] col (dtl*N+n) = A[dtl*128+p, n]
        acols = np.ascontiguousarray(
            Al.reshape(NDT, 128, N).transpose(1, 0, 2).reshape(128, NDT * N))

        # dgw: diag(invG) per (dtl, n): [128, NDT*N*128]
        dgwm = np.zeros((128, NDT * N, 128), np.float32)
        p = np.arange(128)
        for dtl in range(NDT):
            for n in range(N):
                dgwm[p, dtl * N + n, p] = invGl[dtl * 128 + p, n]
        dgw = np.ascontiguousarray(
            dgwm.reshape(128, NDT * N * 128)).astype(np.float16)

        # dskw: diag(D_skip) per dtl
        dskm = np.zeros((128, NDT, 128), np.float32)
        for dtl in range(NDT):
            dskm[p, dtl, p] = D_skip[loc][dtl * 128 + p]
        dskw = np.ascontiguousarray(
            dskm.reshape(128, NDT * 128)).astype(np.float16)

        # qw: [16, NDT*128]  qw[n, dtl*128+p] = invG[dtl*128+p, n]
        qwm = np.ascontiguousarray(
            invGl.T.reshape(N, NDT, 128).reshape(16, NDT * 128)
        ).astype(np.float16)

        # w0init: G*state0 laid out [128, NDT*N]
        w0 = (Gl * state[b, loc]).reshape(NDT, 128, N).transpose(1, 0, 2)
        w0 = np.ascontiguousarray(w0.reshape(128, NDT * N)).astype(np.float32)

        in_maps.append({
            "x16": np.ascontiguousarray(x[b][:, perm]).astype(np.float16),
            "wall": wall,
            "w2r": w2r,
            "bdt2": bd,
            "acols": acols,
            "dgw": dgw,
            "dskw": dskw,
            "qw": qwm,
            "selbc": selbc,
            "nident": nident,
            "ident16": ident16,
            "w0init": w0,
        })

    res = run_bass_kernel_spmd(nc, in_maps, core_ids=list(range(NCORES)))

    y = np.empty((B, T, D), np.float32)
    for c in range(NCORES):
        b, h = c // 2, c % 2
        y[b][:, h * DH:(h + 1) * DH] = res.results[c]["yT"].T.astype(
            np.float32)
    return y
